# revision 1
# baseline (speedup 1.0000x reference)
"""NNConv (edge-conditioned graph conv) Trainium2 kernel, 8-core SPMD.

Strategy: edges are dst-sorted on host and bucketed into 8 contiguous
node ranges (1250 nodes/core), so each core owns a disjoint slice of the
output and no cross-core reduction is needed.  Per core:
  PE      : z = [ea;1]^T @ [W_edge;b]  (per-edge 32x32 weight logits)
  ACT     : relu + f32->bf16 evacuation of PSUM
  GPSIMD  : y = relu(z) * x_src broadcast  (per-edge message terms)
  DVE     : sum_i y[:, (o,i)] -> msg[t, o]  (+ count col = 1)
  DVE     : per-partition prefix scan over the partition-minor edge order
  PE      : strict-lower-triangular matmul for the cross-partition carry
  DMA     : prefix table P -> HBM; indirect row gathers at segment bounds
  PE/DVE  : aggr = (P[e_n]-P[e_{n-1}]) / max(cnt,1) + x@root + bias
"""

import sys
import time

sys.path.insert(0, "/opt/trn_rl_repo")

import numpy as np
import ml_dtypes

import concourse.bass as bass
import concourse.bacc as bacc
import concourse.mybir as mybir
import concourse.tile as tile
from concourse.bass_utils import run_bass_kernel_spmd
import os
STAGE = int(os.environ.get("KSTAGE", "9"))
REPS = int(os.environ.get("KREPS", "1"))
SKIP = set(os.environ.get("KSKIP", "").split(","))
KLVL = int(os.environ.get("KLVL", "5"))

F32 = mybir.dt.float32
BF16 = mybir.dt.bfloat16
I16 = mybir.dt.int16
I32 = mybir.dt.int32
BF = ml_dtypes.bfloat16

# problem constants (hardcoded per the harness contract)
N_NODES = 10000
IN_C = 32
OUT_C = 32
EDGE_F = 16
N_EDGES = 320000
CORES = 8
NPC = N_NODES // CORES          # 1250 nodes per core
NPT = 10                        # node tiles per core (128 each, padded 1280)
NPAD = 128 * NPT
JT = 328                        # free-dim edge slots per partition
EPC = 128 * JT                  # 41984 padded edge slots per core
IDXW = EPC // 16                # 2624
GCH = 41                        # x-gather chunks (1024 idxs each: SWDGE ring cap)
JCH = JT // GCH                 # 82 tiles per chunk
MC = 33                         # msg cols: 32 outputs + count

_CACHE = {}


def _build():
    if "nc" in _CACHE:
        return _CACHE["nc"]
    nc = bacc.Bacc("TRN2", target_bir_lowering=False, debug=False,
                   num_devices=CORES)

    ea_d = nc.declare_dram_parameter("ea", [EDGE_F, EPC], BF16, isOutput=False)
    gidx_d = nc.declare_dram_parameter("gidx", [128, IDXW], I16, isOutput=False)
    xg_d = nc.declare_dram_parameter("xg", [N_NODES, 128], BF16, isOutput=False)
    w_d = nc.declare_dram_parameter("wmat", [EDGE_F + 1, 1024], BF16, isOutput=False)
    bidx_d = nc.declare_dram_parameter("bidx", [128, NPT], I32, isOutput=False)
    pidx_d = nc.declare_dram_parameter("pidx", [128, NPT], I32, isOutput=False)
    xtb_d = nc.declare_dram_parameter("xtb", [IN_C + 1, NPAD], F32, isOutput=False)
    rootb_d = nc.declare_dram_parameter("rootb", [IN_C + 1, OUT_C], F32, isOutput=False)
    tri_d = nc.declare_dram_parameter("tri", [128, 128], F32, isOutput=False)
    out_d = nc.declare_dram_parameter("out", [NPAD, OUT_C], F32, isOutput=True)

    p_hbm = nc.dram_tensor("pfx", [EPC + 128, MC], F32)

    with tile.TileContext(nc) as tc:
        with (
            tc.tile_pool(name="const", bufs=1) as cpool,
            tc.tile_pool(name="big", bufs=1) as bigpool,
            tc.tile_pool(name="xsp", bufs=2) as xspool,
            tc.tile_pool(name="zp", bufs=3, space="PSUM") as zpsum,
            tc.tile_pool(name="work", bufs=3) as wpool,
            tc.tile_pool(name="small", bufs=1) as spool,
            tc.tile_pool(name="sps", bufs=1, space="PSUM") as spsum,
        ):
            # ---- resident tiles ----
            w_t = cpool.tile([EDGE_F + 1, 1024], BF16)
            nc.sync.dma_start(w_t[:], w_d[:])
            ea_t = bigpool.tile([EDGE_F + 1, EPC], BF16)
            nc.gpsimd.memset(ea_t[0:1, :], 1.0)
            nc.sync.dma_start(ea_t[1:, :], ea_d[:])
            gidx_t = cpool.tile([128, IDXW], I16)
            nc.sync.dma_start(gidx_t[:], gidx_d[:])

            msg_t = bigpool.tile([128, JT, MC], F32)
            # count column = 1.0 for every slot
            nc.gpsimd.memset(
                msg_t[:].rearrange("p j c -> p (j c)")[:, OUT_C::MC], 1.0
            )

            # zero row(s) of the prefix table (used by empty-segment bounds)
            zrow = spool.tile([128, MC], F32)
            nc.gpsimd.memset(zrow[:], 0.0)
            nc.sync.dma_start(
                p_hbm[EPC:EPC + 128, :], zrow[:]
            )

            for _rep in range(REPS):
                # ---- main edge loop ----
                for c in range(GCH):
                    xs_t = xspool.tile([128, JCH, 128], BF16)
                    if "gather" not in SKIP:
                      nc.gpsimd.dma_gather(
                        xs_t[:], xg_d[:],
                        gidx_t[:, c * (IDXW // GCH):(c + 1) * (IDXW // GCH)],
                        EPC // GCH, EPC // GCH, 128,
                      )
                    for jj in range(JCH):
                        j = c * JCH + jj
                        if KLVL < 2:
                            continue
                        z_ps = zpsum.tile([128, 1024], F32)
                        for h in range(2):
                            nc.tensor.matmul(
                                z_ps[:, h * 512:(h + 1) * 512],
                                ea_t[:, j * 128:(j + 1) * 128],
                                w_t[:, h * 512:(h + 1) * 512],
                                start=True, stop=True,
                            )
                        if KLVL < 3:
                            continue
                        zr_t = wpool.tile([128, 1024], BF16, tag="zr")
                        nc.scalar.activation(
                            zr_t[:], z_ps[:], mybir.ActivationFunctionType.Relu
                        )
                        if KLVL < 4:
                            continue
                        mult_eng = nc.vector if os.environ.get("KMULT", "gps") == "dve" else nc.gpsimd
                        y_t = wpool.tile([128, 1024], BF16, tag="y")
                        mult_eng.tensor_tensor(
                            y_t[:].rearrange("p (o i) -> p o i", i=IN_C),
                            zr_t[:].rearrange("p (o i) -> p o i", i=IN_C),
                            xs_t[:, jj, 0:IN_C].unsqueeze(1).broadcast_to(
                                [128, OUT_C, IN_C]
                            ),
                            mybir.AluOpType.mult,
                        )
                        if KLVL < 5:
                            continue
                        nc.vector.tensor_reduce(
                            msg_t[:, j, 0:OUT_C],
                            y_t[:].rearrange("p (o i) -> p o i", i=IN_C),
                            mybir.AxisListType.X,
                            mybir.AluOpType.add,
                        )

                # ---- segment sum via prefix scan ----
                if STAGE >= 2:
                  tri_t = cpool.tile([128, 128], F32)
                  nc.sync.dma_start(tri_t[:], tri_d[:])
                  tot_t = spool.tile([128, MC], F32)
                  nc.vector.tensor_reduce(
                      tot_t[:],
                      msg_t[:].rearrange("p j c -> p c j"),
                      mybir.AxisListType.X,
                      mybir.AluOpType.add,
                  )
                  carry_ps = spsum.tile([128, MC], F32)
                  nc.tensor.matmul(carry_ps[:], tri_t[:], tot_t[:], start=True, stop=True)
                  carry_t = spool.tile([128, MC], F32)
                  nc.vector.tensor_copy(carry_t[:], carry_ps[:])

                  zcol = spool.tile([128, 1], F32)
                  nc.gpsimd.memset(zcol[:], 0.0)
                  for cc in range(MC):
                      col = msg_t[:].rearrange("p j c -> p c j")[:, cc, :]
                      nc.vector.tensor_tensor_scan(
                          col, col,
                          zcol[:].broadcast_to([128, JT]),
                          carry_t[:, cc:cc + 1],
                          mybir.AluOpType.add,
                          mybir.AluOpType.add,
                      )

                  nc.sync.dma_start(
                      p_hbm[0:EPC, :].rearrange("(p j) c -> p j c", j=JT), msg_t[:]
                  )

                # ---- boundary gathers + final update ----
                if STAGE >= 3:
                  bidx_t = spool.tile([128, NPT], I32)
                  nc.sync.dma_start(bidx_t[:], bidx_d[:])
                  pidx_t = spool.tile([128, NPT], I32)
                  nc.sync.dma_start(pidx_t[:], pidx_d[:])
                  pb_t = spool.tile([128, NPT, MC], F32)
                  pp_t = spool.tile([128, NPT, MC], F32)
                  for j2 in range(NPT):
                      nc.gpsimd.indirect_dma_start(
                          pb_t[:, j2, :], None, p_hbm[:],
                          bass.IndirectOffsetOnAxis(ap=bidx_t[:, j2:j2 + 1], axis=0),
                      )
                      nc.gpsimd.indirect_dma_start(
                          pp_t[:, j2, :], None, p_hbm[:],
                          bass.IndirectOffsetOnAxis(ap=pidx_t[:, j2:j2 + 1], axis=0),
                      )
                  seg_t = spool.tile([128, NPT, MC], F32)
                  nc.vector.tensor_tensor(
                      seg_t[:], pb_t[:], pp_t[:], mybir.AluOpType.subtract
                  )
                  cnt_t = spool.tile([128, NPT], F32)
                  nc.vector.tensor_scalar_max(
                      cnt_t[:], seg_t[:, :, OUT_C], 1.0
                  )
                  rcp_t = spool.tile([128, NPT], F32)
                  nc.vector.reciprocal(rcp_t[:], cnt_t[:])

                  xtb_t = spool.tile([IN_C + 1, NPAD], F32)
                  nc.sync.dma_start(xtb_t[:], xtb_d[:])
                  rootb_t = spool.tile([IN_C + 1, OUT_C], F32)
                  nc.sync.dma_start(rootb_t[:], rootb_d[:])
                  rx_ps = spsum.tile([128, NPT * OUT_C], F32)
                  for j2 in range(NPT):
                      nc.tensor.matmul(
                          rx_ps[:, j2 * OUT_C:(j2 + 1) * OUT_C],
                          xtb_t[:, j2 * 128:(j2 + 1) * 128],
                          rootb_t[:],
                          start=True, stop=True,
                      )
                  fin_t = spool.tile([128, NPT * OUT_C], F32)
                  for j2 in range(NPT):
                      nc.vector.scalar_tensor_tensor(
                          fin_t[:, j2 * OUT_C:(j2 + 1) * OUT_C],
                          seg_t[:, j2, 0:OUT_C],
                          rcp_t[:, j2:j2 + 1],
                          rx_ps[:, j2 * OUT_C:(j2 + 1) * OUT_C],
                          mybir.AluOpType.mult,
                          mybir.AluOpType.add,
                      )
                  nc.sync.dma_start(
                      out_d[:].rearrange("(j p) o -> p j o", p=128),
                      fin_t[:].rearrange("p (j o) -> p j o", o=OUT_C),
                  )

                if STAGE < 3:
                    nc.sync.dma_start(
                        out_d[:].rearrange("(j p) o -> p j o", p=128),
                        msg_t[:, 0:NPT, 0:OUT_C],
                    )
                elif STAGE < 4:
                    nc.sync.dma_start(
                        out_d[:].rearrange("(j p) o -> p j o", p=128),
                        pb_t[:, :, 0:OUT_C],
                    )

    nc.compile()
    _CACHE["nc"] = nc
    return nc


def _wrap16(v):
    """idx slot s -> [s % 16, s // 16], replicated to 128 partitions."""
    w = np.ascontiguousarray(v.reshape(-1, 16).T)
    return np.tile(w, (8, 1))


def _prep_inputs(x, edge_index, edge_attr, W_edge, b_edge, root, bias):
    """Host-side sharding: dst-sort, bucket by node range, physical layout."""
    src = np.asarray(edge_index[0], dtype=np.int64)
    dst = np.asarray(edge_index[1], dtype=np.int64)
    ea = np.asarray(edge_attr, dtype=np.float32)
    x = np.asarray(x, dtype=np.float32)

    order = np.argsort(dst, kind="stable")
    dst_s = dst[order]
    bounds = np.searchsorted(dst_s, np.arange(CORES + 1) * NPC)

    # shared tensors
    # W col order c' = (o, i); fold bias as extra row
    W = np.asarray(W_edge, dtype=np.float32).reshape(EDGE_F, IN_C, OUT_C)
    b = np.asarray(b_edge, dtype=np.float32).reshape(IN_C, OUT_C)
    wmat = np.empty((EDGE_F + 1, 1024), dtype=np.float32)
    wmat[0] = b.T.reshape(1024)
    wmat[1:] = W.transpose(0, 2, 1).reshape(EDGE_F, 1024)
    wmat = wmat.astype(BF)
    xg = np.tile(x.astype(BF), (1, 4))
    rootb = np.concatenate(
        [np.asarray(root, np.float32),
         np.asarray(bias, np.float32)[None, :]], axis=0
    )
    tri = np.tril(np.ones((128, 128), np.float32), -1).T.copy()  # tri[k,m]=1 if k<m

    s_all = np.arange(EPC)
    t_of_s = (s_all % 128) * JT + s_all // 128
    n_all = np.arange(NPAD)
    ins = []
    for k in range(CORES):
        lo, hi = int(bounds[k]), int(bounds[k + 1])
        m = hi - lo
        assert m <= EPC, f"core {k} edge count {m} > {EPC}"
        ids = order[lo:hi]
        ids_pad = np.concatenate([ids, np.full(EPC - m, ids[0] if m else 0)])
        col_ids = ids_pad[np.minimum(t_of_s, EPC - 1)]
        ea_sh = np.ascontiguousarray(ea[col_ids].T.astype(BF))
        gidx = _wrap16(src[col_ids].astype(np.int16))

        # segment bounds in sorted-local t coordinates
        dloc = dst_s[lo:hi] - k * NPC
        cum = np.searchsorted(dloc, np.arange(NPC + 1))
        e_last = cum[1:] - 1            # last edge of node n (or -1)
        e_prev = cum[:-1] - 1           # last edge of node n-1
        zero_row = EPC                  # zeroed row in p_hbm
        bv = np.where(e_last >= 0, e_last, zero_row)
        pv = np.where(e_prev >= 0, e_prev, zero_row)
        bfull = np.full(NPAD, zero_row, np.int64)
        pfull = np.full(NPAD, zero_row, np.int64)
        bfull[:NPC] = bv
        pfull[:NPC] = pv
        bidx = np.ascontiguousarray(
            bfull.reshape(NPT, 128).T.astype(np.int32))
        pidx = np.ascontiguousarray(
            pfull.reshape(NPT, 128).T.astype(np.int32))

        xtb = np.zeros((IN_C + 1, NPAD), np.float32)
        xtb[:IN_C, :NPC] = x[k * NPC:(k + 1) * NPC].T
        xtb[IN_C] = 1.0

        ins.append({
            "ea": ea_sh, "gidx": gidx, "xg": xg, "wmat": wmat,
            "bidx": bidx, "pidx": pidx, "xtb": xtb, "rootb": rootb,
            "tri": tri,
        })
    return ins


def kernel(**inputs) -> np.ndarray:
    nc = _build()
    ins = _prep_inputs(**inputs)
    res = run_bass_kernel_spmd(nc, ins, list(range(CORES)))
    outs = [res.results[k]["out"][:NPC] for k in range(CORES)]
    return np.concatenate(outs, axis=0)

